# revision 1
# baseline (speedup 1.0000x reference)
"""Trainium2 Bass kernel for nn_DiscreteFlow (masked autoregressive MLP +
per-variable segment reductions), data-parallel over 8 NeuronCores.

Per batch row b (reference semantics):
  one_hot = onehot(x[b])                      # [16*64]
  net_in  = [c[b], one_hot[:960]]             # [976]
  h1 = relu(net_in @ (W0*m0) + b0)
  h2 = relu(h1 @ (W1*m1) + b1)
  logits = h2 @ (W2*m2) + b2                  # [1024]
  out[b] = sum_l logits[64l + x[b,l]] - sum_l log(sum_d exp(logits[64l+d]))

Device layout: activations are feature-major [feature_chunk(128), 8, Btile]
so every GEMM is lhsT=weight-tile (stationary) x rhs=activation [128,512]
(moving).  Hidden units are permuted (sorted by MADE degree) on the host so
the masked weights become block-triangular; all-zero 128x128 weight tiles
are skipped, and the surviving K-chunk pairs run as fp8e4 DoubleRow matmuls
(the [128, KC, *] chunk layout is natively the DoubleRow paired-K 3D AP).
The one-hot is built without touching the PE: an int8 DMA broadcast
(partition-step-0 DRAM source) replicates x across each 64-row block, then
one DVE is_equal against the per-partition d column yields exact {0,1} fp8.
The "chosen logit" gather is (psum logits)*one_hot on DVE followed by a
block-indicator matmul col-packed against the exp block-norm indicator
matmul (psum partitions 0:16 / 32:48, separate banks so both accumulation
groups stay open); out = sum_l chosen_l - ln(norm_l) with a single ACT Ln
and a float32r ones-matmul.  With all-zero biases (always true for this
problem's setup_inputs) every psum evacuation covers two banks (1024 free
elements); a general per-chunk biased path is kept as fallback.
CoreSim cost-model makespan: ~120 us per core.
"""

import numpy as np
import ml_dtypes

import concourse.bass as bass
import concourse.tile as tile
import concourse.mybir as mybir
from concourse.bass_utils import run_bass_kernel_spmd

F32 = mybir.dt.float32
F32R = mybir.dt.float32r
BF16 = mybir.dt.bfloat16
FP8 = mybir.dt.float8e4
fp8 = mybir.dt.np(mybir.dt.float8e4)
AF = mybir.ActivationFunctionType
OP = mybir.AluOpType
bf16 = ml_dtypes.bfloat16

N_CORES = 8
B = 32768
BC = B // N_CORES  # 4096 rows per core
L, DIM, DIMS_C, H = 16, 64, 16, 1024
NET_IN, NET_OUT = 976, 1024
BT = 512           # batch tile (matmul moving free dim)
KC = 8             # feature chunks of 128

# wait-split workaround: this walrus build rejects >1 semaphore wait per
# instruction ("Too many sync wait commands"); push extras onto NOPs.
_MAX_WAITS = 1


def _fix_sync_waits(nc, max_waits=_MAX_WAITS, pe_max_waits=1):
    n_split = 0
    for fn in nc.m.functions:
        for blk in fn.blocks:
            insts = list(blk.instructions)
            new = []
            for inst in insts:
                si = getattr(inst, "sync_info", None)
                waits = list(si.on_wait) if si is not None and si.on_wait else []
                mw = (
                    pe_max_waits
                    if type(inst).__name__ == "InstMatmult"
                    else max_waits
                )
                if len(waits) > mw:
                    extra, keep = waits[:-mw], waits[-mw:]
                    for j in range(0, len(extra), mw):
                        chunk = extra[j : j + mw]
                        nop = mybir.InstNoOp(
                            name=f"{inst.name}-ws{j}",
                            engine=inst.engine,
                            bass_nofuse=True,
                            sync_info=mybir.SyncInfo(on_wait=chunk, on_update=[]),
                        )
                        new.append(nop)
                        n_split += 1
                    si.on_wait = keep
                new.append(inst)
            if len(new) != len(insts):
                blk.instructions = new
    return n_split


def _made_masks():
    in_deg = np.concatenate(
        [np.zeros(DIMS_C, np.int32), np.repeat(np.arange(1, L, dtype=np.int32), DIM)]
    )
    h_deg = np.tile(np.arange(L, dtype=np.int32), H // L + 1)[:H]
    out_deg = np.repeat(np.arange(L, dtype=np.int32), DIM)
    m0 = (h_deg[None, :] >= in_deg[:, None]).astype(np.float32)
    m1 = (h_deg[None, :] >= h_deg[:, None]).astype(np.float32)
    m2 = (out_deg[None, :] >= h_deg[:, None]).astype(np.float32)
    return m0, m1, m2, h_deg


def prep_host(x, c, W0, b0, W1, b1, W2, b2):
    """Host-side weight prep (mask apply + degree-sort permutation + chunked
    device layouts) and input transposes.  Returns (consts, per_core_fn)."""
    m0, m1, m2, h_deg = _made_masks()
    perm = np.argsort(h_deg, kind="stable")

    W0m = (np.asarray(W0, np.float32) * m0)[:, perm]        # [976, 1024]
    W1m = (np.asarray(W1, np.float32) * m1)[perm][:, perm]  # [1024, 1024]
    W2m = (np.asarray(W2, np.float32) * m2)[perm, :]        # [1024, 1024]
    b0p = np.asarray(b0, np.float32)[perm]
    b1p = np.asarray(b1, np.float32)[perm]
    b2f = np.asarray(b2, np.float32)

    w0c = W0m[:DIMS_C].astype(bf16)                         # [16, 1024]
    W0oh = np.zeros((H, H), np.float32)
    W0oh[: NET_IN - DIMS_C] = W0m[DIMS_C:]                  # pad var-15 block w/ 0

    def chunked(w, dt=bf16):  # [1024, 1024] -> [128, 8, 1024]
        return np.ascontiguousarray(
            w.reshape(KC, 128, H).transpose(1, 0, 2)
        ).astype(dt)

    def nzsets(w):
        return [
            [
                ki
                for ki in range(KC)
                if np.any(w[ki * 128 : (ki + 1) * 128, mi * 128 : (mi + 1) * 128])
            ]
            for mi in range(KC)
        ]

    w0c4 = np.zeros((128, H), bf16)
    wxb4r = np.zeros((128, H), bf16)
    consts = {
        "w0c": w0c,
        "w0oh": chunked(W0oh, fp8),
        "w1": chunked(W1m, fp8),
        "w2": chunked(W2m, fp8),
        "b0t": np.ascontiguousarray(b0p.reshape(KC, 128).T),
        "b1t": np.ascontiguousarray(b1p.reshape(KC, 128).T),
        "b2t": np.ascontiguousarray(b2f.reshape(KC, 128).T),
        "ones16": np.ones((L, 1), np.float32),
    }

    # one-hot build: psum[p, b] = x[b, l(p)] (broadcast via block indicator),
    # then onehot = is_equal(psum, d(p)) on DVE.  x, d <= 63: exact in bf16.
    f = np.arange(H)
    lblk = f // DIM
    d = (f % DIM).astype(np.float32)
    wxb = np.zeros((L, H), np.float32)
    wxb[lblk, f] = 1.0
    consts["wxb"] = wxb.astype(bf16)
    consts["dcol"] = np.ascontiguousarray(d.reshape(KC, 128).T)
    consts["dcol8"] = np.ascontiguousarray(
        d.astype(np.int8).reshape(KC, 128).T)
    for j in range(4):
        w0c4[32 * j : 32 * j + L] = w0c
        wxb4r[32 * j : 32 * j + L] = consts["wxb"]
    consts["w0c4"] = w0c4
    consts["wxb4"] = wxb4r
    consts["zero_bias"] = bool(
        not np.any(b0p) and not np.any(b1p) and not np.any(b2f)
    )

    # block indicator: ind[p, ki, l] = 1 if (128 ki + p) // 64 == l
    ind = np.zeros((128, KC, L), np.float32)
    for ki in range(KC):
        g = (128 * ki + np.arange(128)) // DIM
        ind[np.arange(128), ki, g] = 1.0
    consts["ind"] = ind.astype(bf16)
    consts["ind8"] = ind.astype(fp8)

    xTb = np.ascontiguousarray(np.asarray(x, np.float32).T).astype(bf16)  # [16, B]
    xT8 = np.ascontiguousarray(np.asarray(x).T).astype(np.int8)           # [16, B]
    cT = np.ascontiguousarray(np.asarray(c, np.float32).T).astype(bf16)
    B_ = xTb.shape[1]
    x4 = np.zeros((128, B_), bf16)
    c4 = np.zeros((128, B_), bf16)
    for j in range(4):
        x4[32 * j : 32 * j + L] = xTb
        c4[32 * j : 32 * j + L] = cT

    def per_core(ci, bc):
        s = slice(ci * bc, (ci + 1) * bc)
        return {"xT": np.ascontiguousarray(xTb[:, s]),
                "xT8": np.ascontiguousarray(xT8[:, s]),
                "cT": np.ascontiguousarray(cT[:, s]),
                "x4": np.ascontiguousarray(x4[:, s]),
                "c4": np.ascontiguousarray(c4[:, s])}

    return consts, per_core, nzsets(W0oh), nzsets(W1m), nzsets(W2m)


def build_program_biased(nz0, nz1, nz2, bc=BC, repeat=1):
    """General path: per-chunk evacuations with fused per-feature biases."""
    nt = bc // BT
    nc = bass.Bass()

    xT_d = nc.dram_tensor("xT", [L, bc], BF16, kind="ExternalInput")
    cT_d = nc.dram_tensor("cT", [L, bc], BF16, kind="ExternalInput")
    w0c_d = nc.dram_tensor("w0c", [DIMS_C, H], BF16, kind="ExternalInput")
    w0oh_d = nc.dram_tensor("w0oh", [128, KC, H], FP8, kind="ExternalInput")
    w1_d = nc.dram_tensor("w1", [128, KC, H], FP8, kind="ExternalInput")
    w2_d = nc.dram_tensor("w2", [128, KC, H], FP8, kind="ExternalInput")
    wxb_d = nc.dram_tensor("wxb", [L, H], BF16, kind="ExternalInput")
    dcol_d = nc.dram_tensor("dcol", [128, KC], F32, kind="ExternalInput")
    b0t_d = nc.dram_tensor("b0t", [128, KC], F32, kind="ExternalInput")
    b1t_d = nc.dram_tensor("b1t", [128, KC], F32, kind="ExternalInput")
    b2t_d = nc.dram_tensor("b2t", [128, KC], F32, kind="ExternalInput")
    ind_d = nc.dram_tensor("ind", [128, KC, L], BF16, kind="ExternalInput")
    ones_d = nc.dram_tensor("ones16", [L, 1], F32R, kind="ExternalInput")
    out_d = nc.dram_tensor("out", [1, bc], F32, kind="ExternalOutput")

    # which relu evacuations run on ACT (rest on DVE), to balance engines:
    # per btile ACT has 8 exp (+ln/copy), DVE has 8 chosen-mul + 8 is_equal.
    relu_on_act = {(0, mi) for mi in range(KC)} | {(1, 0), (1, 1), (1, 2)}

    with tile.TileContext(nc) as tc:
        with (
            tc.tile_pool(name="consts", bufs=1) as cpool,
            tc.tile_pool(name="work", bufs=2) as wpool,
            tc.tile_pool(name="small", bufs=2) as spool,
            tc.tile_pool(name="pp", bufs=5, space="PSUM") as pp,
            tc.tile_pool(name="pnc", bufs=1, space="PSUM") as pnc,
            tc.tile_pool(name="pout", bufs=1, space="PSUM") as pout,
        ):
            w0c_sb = cpool.tile([DIMS_C, H], BF16)
            nc.sync.dma_start(w0c_sb[:], w0c_d[:])
            w0oh_sb = cpool.tile([128, KC, H], FP8)
            nc.sync.dma_start(w0oh_sb[:, 0:2, :], w0oh_d[:, 0:2, :])
            nc.sync.dma_start(w0oh_sb[:, 2:, :], w0oh_d[:, 2:, :])
            w1_sb = cpool.tile([128, KC, H], FP8)
            nc.sync.dma_start(w1_sb[:, 0:2, :], w1_d[:, 0:2, :])
            w2_sb = cpool.tile([128, KC, H], FP8)
            nc.sync.dma_start(w2_sb[:, 0:2, :], w2_d[:, 0:2, :])
            nc.sync.dma_start(w1_sb[:, 2:, :], w1_d[:, 2:, :])
            nc.sync.dma_start(w2_sb[:, 2:, :], w2_d[:, 2:, :])
            wxb_sb = cpool.tile([L, H], BF16)
            nc.sync.dma_start(wxb_sb[:], wxb_d[:])
            dcol_sb = cpool.tile([128, KC], F32)
            nc.sync.dma_start(dcol_sb[:], dcol_d[:])
            b0_sb = cpool.tile([128, KC], F32)
            nc.sync.dma_start(b0_sb[:], b0t_d[:])
            b1_sb = cpool.tile([128, KC], F32)
            nc.sync.dma_start(b1_sb[:], b1t_d[:])
            b2_sb = cpool.tile([128, KC], F32)
            nc.sync.dma_start(b2_sb[:], b2t_d[:])
            ind_sb = cpool.tile([128, KC, L], BF16)
            nc.sync.dma_start(ind_sb[:], ind_d[:])
            ones_sb = cpool.tile([L, 1], F32R)
            nc.sync.dma_start(ones_sb[:], ones_d[:])
            if bc > BT:
                nc.sync.dma_start(ct4_sb[:, BT:], c4_d[:, BT:])
            ct_sb = cpool.tile([L, bc], BF16)
            nc.sync.dma_start(ct_sb[:], cT_d[:])
            xb_sb = cpool.tile([L, bc], BF16)
            nc.sync.dma_start(xb_sb[:], xT_d[:])

            def relu_evac(layer, mi, dst, ps, bias_col):
                if (layer, mi) in relu_on_act:
                    nc.scalar.activation(dst, ps, AF.Relu, bias=bias_col)
                else:
                    nc.vector.tensor_scalar(
                        dst, ps, bias_col, 0.0, op0=OP.add, op1=OP.max
                    )

            for bt in range(nt * repeat):
                bts = slice((bt % nt) * BT, (bt % nt + 1) * BT)

                # ---- one-hot build: psum = x bcast, oh = (psum == d(p))
                oh = wpool.tile([128, KC, BT], FP8)
                for ki in range(KC):
                    ps = pp.tile([128, BT], F32)
                    nc.tensor.matmul(
                        ps[:],
                        wxb_sb[:, ki * 128 : (ki + 1) * 128],
                        xb_sb[:, bts],
                        start=True,
                        stop=True,
                    )
                    nc.vector.tensor_scalar(
                        oh[:, ki, :], ps[:], dcol_sb[:, ki : ki + 1], None,
                        op0=OP.is_equal,
                    )

                # ---- layer 0: h1 = relu(c @ W0c + onehot @ W0oh + b0)
                h1 = wpool.tile([128, KC, BT], FP8)
                for mi in range(KC):
                    ps = pp.tile([128, BT], F32)
                    nc.tensor.matmul(
                        ps[:],
                        w0c_sb[:, mi * 128 : (mi + 1) * 128],
                        ct_sb[:, bts],
                        start=True,
                        stop=False,
                    )
                    kp = sorted({ki // 2 for ki in nz0[mi]})
                    for idx, k2 in enumerate(kp):
                        nc.tensor.matmul(
                            ps[:],
                            w0oh_sb[:, 2 * k2 : 2 * k2 + 2, mi * 128 : (mi + 1) * 128],
                            oh[:, 2 * k2 : 2 * k2 + 2, :],
                            start=False,
                            stop=(idx == len(kp) - 1),
                            perf_mode=mybir.MatmulPerfMode.DoubleRow,
                        )
                    relu_evac(0, mi, h1[:, mi, :], ps[:], b0_sb[:, mi : mi + 1])

                # ---- layer 1
                h2 = wpool.tile([128, KC, BT], FP8)
                for mi in range(KC):
                    ps = pp.tile([128, BT], F32)
                    kp = sorted({ki // 2 for ki in nz1[mi]})
                    for idx, k2 in enumerate(kp):
                        nc.tensor.matmul(
                            ps[:],
                            w1_sb[:, 2 * k2 : 2 * k2 + 2, mi * 128 : (mi + 1) * 128],
                            h1[:, 2 * k2 : 2 * k2 + 2, :],
                            start=(idx == 0),
                            stop=(idx == len(kp) - 1),
                            perf_mode=mybir.MatmulPerfMode.DoubleRow,
                        )
                    relu_evac(1, mi, h2[:, mi, :], ps[:], b1_sb[:, mi : mi + 1])

                # ---- layer 2; evac exp(logits+b2), then masked exp
                # (all-bf16 SBUF tensor_tensor -> DVE 2x mode)
                expc = wpool.tile([128, KC, BT], BF16)
                chm = wpool.tile([128, KC, BT], BF16)
                for mi in range(KC):
                    ps = pp.tile([128, BT], F32)
                    kp = sorted({ki // 2 for ki in nz2[mi]})
                    for idx, k2 in enumerate(kp):
                        nc.tensor.matmul(
                            ps[:],
                            w2_sb[:, 2 * k2 : 2 * k2 + 2, mi * 128 : (mi + 1) * 128],
                            h2[:, 2 * k2 : 2 * k2 + 2, :],
                            start=(idx == 0),
                            stop=(idx == len(kp) - 1),
                            perf_mode=mybir.MatmulPerfMode.DoubleRow,
                        )
                    nc.scalar.activation(
                        expc[:, mi, :], ps[:], AF.Exp, bias=b2_sb[:, mi : mi + 1]
                    )
                    nc.vector.tensor_mul(chm[:, mi, :], expc[:, mi, :], oh[:, mi, :])

                # ---- per-variable block sums via indicator matmul
                pn = pnc.tile([L, BT], F32)
                for ki in range(KC):
                    nc.tensor.matmul(
                        pn[:], ind_sb[:, ki, :], expc[:, ki, :],
                        start=(ki == 0), stop=(ki == KC - 1),
                    )
                pc2 = pnc.tile([L, BT], F32)
                for ki in range(KC):
                    nc.tensor.matmul(
                        pc2[:], ind_sb[:, ki, :], chm[:, ki, :],
                        start=(ki == 0), stop=(ki == KC - 1),
                    )

                # ---- out = sum_l (ln exp(chosen)_l - ln norm_l)
                lnn = spool.tile([L, BT], F32)
                nc.scalar.activation(lnn[:], pn[:], AF.Ln)
                lnc = spool.tile([L, BT], F32)
                nc.scalar.activation(lnc[:], pc2[:], AF.Ln)
                dif = spool.tile([L, BT], F32R)
                nc.vector.tensor_sub(dif[:], lnc[:], lnn[:])
                po = pnc.tile([1, BT], F32)
                nc.tensor.matmul(po[:], ones_sb[:], dif[:], start=True, stop=True)
                ot = spool.tile([1, BT], F32)
                nc.scalar.activation(ot[:], po[:], AF.Copy)
                nc.sync.dma_start(out_d[0:1, bts], ot[:])

    return nc


_CACHE = {}


def _get_program(bc=BC):
    if bc not in _CACHE:
        _CACHE[bc] = None  # placeholder; program depends on nz sets -> built in kernel
    return _CACHE[bc]


def kernel(x, c, W0, b0, W1, b1, W2, b2):
    x = np.asarray(x)
    c = np.asarray(c)
    assert x.shape == (B, L) and c.shape == (B, DIMS_C), (
        f"kernel compiled for x[{B},{L}]/c[{B},{DIMS_C}], got "
        f"{x.shape}/{c.shape}"
    )
    consts, per_core, nz0, nz1, nz2 = prep_host(x, c, W0, b0, W1, b1, W2, b2)

    zero_bias = consts.pop("zero_bias")
    key = ("prog", BC, zero_bias)
    if key not in _CACHE:
        builder = build_program_paired if zero_bias else build_program_biased
        nc_new = builder(nz0, nz1, nz2, BC)
        _fix_sync_waits(nc_new)
        _CACHE[key] = nc_new
    nc = _CACHE[key]

    used = {
        a.memorylocations[0].name
        for fn in nc.m.functions
        for a in fn.allocations
        if isinstance(a, mybir.MemoryLocationSet) and a.kind == "ExternalInput"
    }
    consts = {k: v for k, v in consts.items() if k in used}

    in_maps = []
    for ci in range(N_CORES):
        m = dict(consts)
        m.update(per_core(ci, BC))
        in_maps.append({k: v for k, v in m.items() if k in used})

    _CACHE["last_run"] = (nc, in_maps)
    res = None
    for attempt in range(3):
        try:
            res = run_bass_kernel_spmd(
                nc, in_maps, core_ids=list(range(N_CORES))
            )
            break
        except Exception:
            # transient NRT_EXEC_UNIT_UNRECOVERABLE from a previously
            # wedged device clears on retry
            if attempt == 2:
                raise
            import time as _time

            _time.sleep(2.0)
    out = np.concatenate([res.results[ci]["out"][0] for ci in range(N_CORES)])
    return out.astype(np.float32)


if __name__ == "__main__":
    rng = np.random.default_rng(0)
    x = rng.integers(0, DIM, size=(B, L)).astype(np.int32)
    c = rng.standard_normal((B, DIMS_C), dtype=np.float32)
    s0 = 1.0 / np.sqrt(NET_IN)
    s1 = 1.0 / np.sqrt(H)
    W0 = rng.standard_normal((NET_IN, H), dtype=np.float32) * s0
    W1 = rng.standard_normal((H, H), dtype=np.float32) * s1
    W2 = rng.standard_normal((H, NET_OUT), dtype=np.float32) * s1
    b0 = np.zeros(H, np.float32)
    b1 = np.zeros(H, np.float32)
    b2 = np.zeros(NET_OUT, np.float32)
    out = kernel(x, c, W0, b0, W1, b1, W2, b2)
    print(out.shape, out[:8])


def build_program_paired(nz0, nz1, nz2, bc=BC, repeat=1):
    """Fast path for all-zero biases: pair psum banks so every evacuation
    covers 1024 free elements (half the elementwise ops), row-pack the
    K=16 matmuls (x-broadcast + c-part) four to the PE array, and
    col-pack the two block-indicator reductions into one psum bank."""
    nt = bc // BT
    nc = bass.Bass()

    xT_d = nc.dram_tensor("xT8", [L, bc], mybir.dt.int8, kind="ExternalInput")
    c4_d = nc.dram_tensor("c4", [128, bc], BF16, kind="ExternalInput")
    w0c4_d = nc.dram_tensor("w0c4", [128, H], BF16, kind="ExternalInput")
    w0oh_d = nc.dram_tensor("w0oh", [128, KC, H], FP8, kind="ExternalInput")
    w1_d = nc.dram_tensor("w1", [128, KC, H], FP8, kind="ExternalInput")
    w2_d = nc.dram_tensor("w2", [128, KC, H], FP8, kind="ExternalInput")
    dcol_d = nc.dram_tensor("dcol", [128, KC], F32, kind="ExternalInput")
    ind_d = nc.dram_tensor("ind", [128, KC, L], BF16, kind="ExternalInput")
    ones_d = nc.dram_tensor("ones16", [L, 1], F32R, kind="ExternalInput")
    out_d = nc.dram_tensor("out", [1, bc], F32, kind="ExternalOutput")

    # paired-relu evacuations assigned to ACT (rest DVE), to balance engines
    relu_on_act = {(0, 0), (0, 1), (0, 2), (0, 3), (1, 0)}

    # pair up the nonzero K-chunk sets for DoubleRow (chunks 2k,2k+1 fused)
    nzp = [
        [sorted({ki // 2 for ki in nz[mi]}) for mi in range(KC)]
        for nz in (nz0, nz1, nz2)
    ]

    with tile.TileContext(nc) as tc:
        with (
            tc.tile_pool(name="consts", bufs=1) as cpool,
            tc.tile_pool(name="work", bufs=3) as wpool,
            tc.tile_pool(name="small", bufs=2) as spool,
            tc.tile_pool(name="pp", bufs=3, space="PSUM") as pp,
            tc.tile_pool(name="pnc", bufs=1, space="PSUM") as pnc,
        ):
            # DMA order matches first-use order so btile 0 starts early:
            # one-hot needs xT/dcol, then L0 needs c/W0, then W1, W2.
            dcol_sb = cpool.tile([128, KC], F32)
            nc.sync.dma_start(dcol_sb[:], dcol_d[:])
            # btile-0 slice of c first so the PE's first matmuls start early
            ct4_sb = cpool.tile([128, bc], BF16)
            nc.sync.dma_start(ct4_sb[:, 0:BT], c4_d[:, 0:BT])
            w0c4_sb = cpool.tile([128, H], BF16)
            nc.sync.dma_start(w0c4_sb[:], w0c4_d[:])
            w0oh_sb = cpool.tile([128, KC, H], FP8)
            nc.sync.dma_start(w0oh_sb[:, 0:2, :], w0oh_d[:, 0:2, :])
            nc.sync.dma_start(w0oh_sb[:, 2:, :], w0oh_d[:, 2:, :])
            w1_sb = cpool.tile([128, KC, H], FP8)
            nc.sync.dma_start(w1_sb[:, 0:2, :], w1_d[:, 0:2, :])
            w2_sb = cpool.tile([128, KC, H], FP8)
            nc.sync.dma_start(w2_sb[:, 0:2, :], w2_d[:, 0:2, :])
            nc.sync.dma_start(w1_sb[:, 2:, :], w1_d[:, 2:, :])
            nc.sync.dma_start(w2_sb[:, 2:, :], w2_d[:, 2:, :])
            ind_sb = cpool.tile([128, KC, L], BF16)
            nc.sync.dma_start(ind_sb[:], ind_d[:])
            ones_sb = cpool.tile([L, 1], F32R)
            nc.sync.dma_start(ones_sb[:], ones_d[:])
            if bc > BT:
                nc.sync.dma_start(ct4_sb[:, BT:], c4_d[:, BT:])

            def pack4(lhs4, rhs4, outs, cols, stop=True):
                # 4 concurrent K=16 matmuls on row groups 0/32/64/96
                for j, (ot, cs) in enumerate(zip(outs, cols)):
                    p = 32 * j
                    nc.tensor.matmul(
                        ot,
                        lhs4[p : p + L, cs],
                        rhs4[p : p + L, :],
                        start=True,
                        stop=stop,
                        tile_position=(p, 0),
                    )

            for bt in range(nt * repeat):
                bts = slice((bt % nt) * BT, (bt % nt + 1) * BT)

                # ---- one-hot: DMA-broadcast x rows across each 64-row
                # block (DRAM source, partition-step-0 AP): partition half h
                # of chunk ki gets xT row 2ki+h; then oh = (x == d(p))
                oh = wpool.tile([128, KC, BT], FP8)
                xbc = wpool.tile([128, KC, BT], mybir.dt.int8, bufs=4)
                xfull = xT_d[:, :]
                for h in range(2):
                    src_ap = bass.AP(
                        tensor=xfull.tensor,
                        offset=h * bc + bt * BT,
                        ap=[[0, DIM], [2 * bc, KC], [1, BT]],
                    )
                    nc.gpsimd.dma_start(
                        xbc[DIM * h : DIM * (h + 1), :, :], src_ap
                    )
                for k2 in range(4):
                    nc.vector.tensor_scalar(
                        oh[:, 2 * k2 : 2 * k2 + 2, :],
                        xbc[:, 2 * k2 : 2 * k2 + 2, :],
                        dcol_sb[:, 0:1], None,
                        op0=OP.is_equal,
                    )

                def emit_group(w_sb, nzp_l, rhs_tile, dst, lid, evac, m2):
                    # one paired mi group; c-part only for layer 0
                    pd = pp.tile([128, 2, BT], F32, tag="ps")
                    if lid == 0:
                        pack4(
                            w0c4_sb,
                            ct4_sb[:, bts],
                            [pd[:, 0, :], pd[:, 1, :]],
                            [
                                slice((2 * m2) * 128, (2 * m2 + 1) * 128),
                                slice((2 * m2 + 1) * 128, (2 * m2 + 2) * 128),
                            ],
                            stop=False,
                        )
                    for j2 in range(2):
                        mi = 2 * m2 + j2
                        kp = nzp_l[mi]
                        for idx, k2 in enumerate(kp):
                            nc.tensor.matmul(
                                pd[:, j2, :],
                                w_sb[:, 2 * k2 : 2 * k2 + 2,
                                     mi * 128 : (mi + 1) * 128],
                                rhs_tile[:, 2 * k2 : 2 * k2 + 2, :],
                                start=(idx == 0 and lid != 0),
                                stop=(idx == len(kp) - 1),
                                perf_mode=mybir.MatmulPerfMode.DoubleRow,
                            )
                    evac(m2, pd, dst)

                def layer(w_sb, nzp_l, rhs_tile, dst, lid, evac):
                    for m2 in range(4):
                        emit_group(w_sb, nzp_l, rhs_tile, dst, lid, evac, m2)

                def relu_evac(lid):
                    def f(m2, pd, dst):
                        if (lid, m2) in relu_on_act:
                            nc.scalar.activation(
                                dst[:, 2 * m2 : 2 * m2 + 2, :], pd[:, :, :], AF.Relu
                            )
                        else:
                            nc.vector.tensor_scalar(
                                dst[:, 2 * m2 : 2 * m2 + 2, :], pd[:, :, :],
                                0.0, None, op0=OP.max,
                            )
                    return f

                # ---- layers 0/1 allocated up front; emission wavefront
                # below interleaves the three layers' mi-pair groups
                h1 = wpool.tile([128, KC, BT], FP8)
                h2 = wpool.tile([128, KC, BT], FP8)

                # ---- layer 2: exp evac in fp8 (norms average away fp8
                # noise), chosen-logit mask straight from psum (keeps full
                # bf16 logit precision), block-sum matmuls interleaved.
                # Norms ride one fp8 DoubleRow matmul per pair; chosen stays
                # bf16.  Separate psum banks so both groups stay open;
                # col-packed at array columns 0:16 and 32:48.
                expc = wpool.tile([128, KC, BT], BF16)
                chm = wpool.tile([128, KC, BT], BF16)
                pnorm = pnc.tile([L, BT], F32)
                pchos = pnc.tile([48, BT], F32)

                def l2_evac(m2, pd, dst):
                    nc.scalar.activation(
                        dst[:, 2 * m2 : 2 * m2 + 2, :], pd[:, :, :], AF.Exp
                    )
                    nc.vector.scalar_tensor_tensor(
                        chm[:, 2 * m2 : 2 * m2 + 2, :],
                        pd[:, :, :],
                        0.0,
                        oh[:, 2 * m2 : 2 * m2 + 2, :],
                        op0=OP.add,
                        op1=OP.mult,
                    )
                    for ki in (2 * m2, 2 * m2 + 1):
                        nc.tensor.matmul(
                            pnorm[:], ind_sb[:, ki, :], expc[:, ki, :],
                            start=(ki == 0), stop=(ki == KC - 1),
                            tile_position=(0, 0),
                        )
                        nc.tensor.matmul(
                            pchos[32 : 32 + L, :], ind_sb[:, ki, :], chm[:, ki, :],
                            start=(ki == 0), stop=(ki == KC - 1),
                            tile_position=(0, 32),
                        )

                # wavefront: group (lid, m2) depends on (lid-1, m2), so
                # interleaving keeps the PE fed while evacuations trail
                emitters = (
                    lambda m2: emit_group(w0oh_sb, nzp[0], oh, h1, 0,
                                          relu_evac(0), m2),
                    lambda m2: emit_group(w1_sb, nzp[1], h1, h2, 1,
                                          relu_evac(1), m2),
                    lambda m2: emit_group(w2_sb, nzp[2], h2, expc, 2,
                                          l2_evac, m2),
                )
                for lid, m2 in ((0, 0), (0, 1), (0, 2), (0, 3),
                                (1, 0), (1, 1), (1, 2), (1, 3),
                                (2, 0), (2, 1), (2, 2), (2, 3)):
                    emitters[lid](m2)

                # ---- out = sum_l (ln exp(chosen)_l - ln norm_l)
                lnn = spool.tile([L, BT], F32)
                nc.scalar.activation(lnn[:], pnorm[:], AF.Ln)
                dif = spool.tile([L, BT], F32R)
                nc.vector.tensor_sub(dif[:], pchos[32 : 32 + L, :], lnn[:])
                # final 16->1 reduce reuses the norms bank after it closes
                nc.tensor.matmul(
                    pnorm[0:1, :], ones_sb[:], dif[:], start=True, stop=True
                )
                ot = spool.tile([1, BT], F32)
                nc.scalar.activation(ot[:], pnorm[0:1, :], AF.Copy)
                nc.sync.dma_start(out_d[0:1, bts], ot[:])

    return nc

